# revision 1
# baseline (speedup 1.0000x reference)
"""Multi-head attention (B=4, N=1024, D=1024, 16 heads x 64) on 8 TRN2 cores.

Sharding: core c -> (batch b = c//2, head-group g = c%2). Each core computes
attention for 8 heads of one batch plus its slice of the output projection
(Wo row-parallel); host sums the two head-group partials per batch.

Device-side math is done fully transposed (feature dim on partitions):
  QT = (Wq_g*scale)^T @ x_b^T          [512, 1024]
  KT = Wk_g^T @ x_b^T                  [512, 1024]
  V  = x_b @ Wv_g                      [1024, 512]   (token dim on partitions)
  per head h (2 packed per PE pass via partition halves):
    ST[j,i] = K Q^T   (K=64 contraction)
    P = exp(ST) * binmT                (mask==0 keep; exp on ScalarE)
    U'[65, i] = [V_h | 1]^T @ P        (row 64 = softmax denominator)
    evac U' to SBUF (value rows on ScalarE, denominator row on DVE — frees
    the PSUM bank after two quick hops), then recip + partition_broadcast
    (GpSimd) + normalize multiply (DVE), all off PSUM
  outT_partial = Wo_g^T-contraction @ OT  [1024, 1024]
Host: out[b] = (outT[2b] + outT[2b+1]).T + bo

Schedule: one flat software pipeline over all (i, pair, jc) steps — each
PV runs one jc behind its ST so exp+mask latency is always covered,
including across block boundaries. V projection first (its DMA deps land
first); QK projections prefetch inside the i0 sweep hooks; the i0 output
projection interleaves into the i1 sweep; normalize chains are emitted two
steps late so they queue behind the next block's masks on DVE.

Hardware lessons baked in (CoreSim won't show these):
  - gpsimd tensor ops (other than partition_broadcast) are ~10x slower
    than the cost model claims — keep tensor_mul off Pool.
  - reciprocal_approx_fast silently reads partition 0 regardless of its
    input AP's partition offset — feed it a partition-0 tile.
  - engine APs must start at 32-aligned partitions (walrus birverifier).
"""

import os
from contextlib import ExitStack, nullcontext

import ml_dtypes
import numpy as np

import concourse.bass as bass
import concourse.mybir as mybir
import concourse.tile as tile
from concourse import bacc
from concourse.bass_utils import run_bass_kernel_spmd

B, N, D = 4, 1024, 1024
HEADS, DH = 16, 64
SCALE = DH ** -0.5
NCORES = 8
HPC = HEADS // 2          # heads per core = 8
IPC = HPC * DH            # inner slice per core = 512
P = 128
IC = 512                  # i (query position) chunk = max psum free dim
NI = N // IC              # 2
NJ = N // P               # 8 key-position chunks
NKC = D // P              # 8 contraction chunks for projections
NMC = IPC // P            # 4 inner chunks per core
NDO = D // P              # 8 output-dim chunks

F32 = mybir.dt.float32
F32R = mybir.dt.float32r
BF16 = mybir.dt.bfloat16
EXP = mybir.ActivationFunctionType.Exp


def _chain(f, g):
    if f is None:
        return g
    def h():
        f()
        g()
    return h


def _r(ap):
    return ap if ap.dtype == F32R else ap.bitcast(F32R)


def _build(loop=1):
    nc = bacc.Bacc("TRN2", target_bir_lowering=False, debug=False)
    xT = nc.dram_tensor("xT", [D, N], BF16, kind="ExternalInput")
    wq = nc.dram_tensor("wq", [D, IPC], BF16, kind="ExternalInput")
    wk = nc.dram_tensor("wk", [D, IPC], BF16, kind="ExternalInput")
    wv = nc.dram_tensor("wv", [D, IPC], BF16, kind="ExternalInput")
    wo = nc.dram_tensor("wo", [IPC, D], F32R, kind="ExternalInput")
    binmT = nc.dram_tensor("binmT", [N, N], BF16, kind="ExternalInput")
    outT = nc.dram_tensor("outT", [D, N], F32, kind="ExternalOutput")
    _dbg = os.environ.get("ATTN_DEBUG", "0") == "1"
    if _dbg:
        dbg_u = nc.dram_tensor("dbg_u", [2, P, IC], F32, kind="ExternalOutput")
        dbg_ot = nc.dram_tensor("dbg_ot", [2, P, NMC, IC], F32, kind="ExternalOutput")
        dbg_rr = nc.dram_tensor("dbg_rr", [2, 1, IC], F32, kind="ExternalOutput")
        dbg_bc = nc.dram_tensor("dbg_bc", [2, DH, IC], F32, kind="ExternalOutput")

    xT_r = xT.rearrange("(kc p) n -> kc p n", p=P)
    wq_r = wq.rearrange("(kc p) m -> kc p m", p=P)
    wk_r = wk.rearrange("(kc p) m -> kc p m", p=P)
    wv_r = wv.rearrange("(kc p) m -> kc p m", p=P)
    wo_r = wo.rearrange("(kc p) m -> kc p m", p=P)
    binmT_r = binmT.rearrange("(jc p) i -> p jc i", p=P)
    outT_r = outT.rearrange("(do p) n -> do p n", p=P)

    with tile.TileContext(nc) as tc, ExitStack() as ctx:
        xt_pool = ctx.enter_context(tc.tile_pool(name="xt", bufs=int(os.environ.get("ATTN_XTB", "1"))))
        w_pool = ctx.enter_context(tc.tile_pool(name="w", bufs=int(os.environ.get("ATTN_WB", "3"))))
        qk_pool = ctx.enter_context(tc.tile_pool(name="qk", bufs=int(os.environ.get("ATTN_QKB", "2"))))
        v_pool = ctx.enter_context(tc.tile_pool(name="v", bufs=int(os.environ.get("ATTN_VB", "2"))))
        m_pool = ctx.enter_context(tc.tile_pool(name="m", bufs=int(os.environ.get("ATTN_MB", "1"))))
        p_pool = ctx.enter_context(tc.tile_pool(name="p", bufs=int(os.environ.get("ATTN_PBUFS", "6"))))
        ot_pool = ctx.enter_context(tc.tile_pool(name="ot", bufs=2))
        stage_pool = ctx.enter_context(tc.tile_pool(name="stage", bufs=3))
        small_pool = ctx.enter_context(tc.tile_pool(name="small", bufs=4))
        bc_pool = ctx.enter_context(tc.tile_pool(name="bc", bufs=2))
        us_pool = ctx.enter_context(tc.tile_pool(name="us", bufs=4))
        _stb = int(os.environ.get("ATTN_STB", "2"))
        psum_pp = ctx.enter_context(
            tc.tile_pool(name="pp", bufs=8 - 2 * _stb, space="PSUM")
        )
        psum_st = ctx.enter_context(tc.tile_pool(name="st", bufs=_stb, space="PSUM"))
        psum_u = psum_pp

        if loop > 1:
            loop_cm = tc.For_i(0, loop, 1)
        else:
            loop_cm = None
        with (loop_cm if loop_cm is not None else nullcontext()):
            # ---------------- phase 0: DMA in ----------------
            # ordered by first use: xt (everything), wv (V proj), wq/wk
            # (QK proj), binm (attention masks), wo later (out proj).
            xt_s = xt_pool.tile([P, NKC, N], BF16, tag="xt")
            wq_s = w_pool.tile([P, NKC, IPC], BF16, tag="w")
            wk_s = w_pool.tile([P, NKC, IPC], BF16, tag="w")
            wv_s = w_pool.tile([P, NKC, IPC], BF16, tag="w")
            # DMA order matches first use: the first half of xt columns plus
            # wv gets the V projection started earliest; binm chunk 0 must
            # beat the first attention mask multiply.
            binm_s = m_pool.tile([P, NJ, N], BF16, tag="binm")
            if os.environ.get("ATTN_DMABIG", "0") == "1":
                # one strided DMA per tensor phase — measured WORSE (192us
                # vs 175): a single dma_start runs on one DMA engine, losing
                # the multi-engine parallelism of per-chunk issues
                xT_p = xT.rearrange("(kc p) n -> p kc n", p=P)
                nc.sync.dma_start(xt_s[:, :, 0:IC], xT_p[:, :, 0:IC])
                nc.sync.dma_start(wv_s, wv.rearrange("(kc p) m -> p kc m", p=P))
                nc.sync.dma_start(xt_s[:, :, IC:N], xT_p[:, :, IC:N])
                nc.sync.dma_start(wq_s, wq.rearrange("(kc p) m -> p kc m", p=P))
                nc.sync.dma_start(binm_s[:, 0, :], binmT_r[:, 0, :])
                nc.sync.dma_start(wk_s, wk.rearrange("(kc p) m -> p kc m", p=P))
                nc.sync.dma_start(binm_s[:, 1:NJ, :], binmT_r[:, 1:NJ, :])
            else:
                # alternate input DMA issues between the SP and ScalarE HWDGE
                # queues: each dma_start costs ~565-667ns of serial sequencer
                # issue, and ScalarE is idle until the first exp (~13us in),
                # so splitting halves the issue serialization at the head
                if os.environ.get("ATTN_DMASPLIT", "1") == "1":
                    issuers = [nc.sync, nc.scalar]
                else:
                    issuers = [nc.sync, nc.sync]
                for kc in range(NKC):
                    issuers[kc % 2].dma_start(xt_s[:, kc, 0:IC], xT_r[kc][:, 0:IC])
                for kc in range(NKC):
                    issuers[kc % 2].dma_start(wv_s[:, kc, :], wv_r[kc])
                for kc in range(NKC):
                    issuers[kc % 2].dma_start(xt_s[:, kc, IC:N], xT_r[kc][:, IC:N])
                for kc in range(NKC):
                    issuers[kc % 2].dma_start(wq_s[:, kc, :], wq_r[kc])
                nc.sync.dma_start(binm_s[:, 0, :], binmT_r[:, 0, :])
                for kc in range(NKC):
                    nc.sync.dma_start(wk_s[:, kc, :], wk_r[kc])
                for jc in range(1, NJ):
                    nc.sync.dma_start(binm_s[:, jc, :], binmT_r[:, jc, :])

            # ---------------- phase 1: projections ----------------
            qt_s = qk_pool.tile([P, NMC, N], BF16, tag="qt")
            kt_s = qk_pool.tile([P, NMC, N], BF16, tag="kt")
            v_s = v_pool.tile([P, NJ, HPC, DH + 1], BF16, tag="v")
            ones_col = small_pool.tile([P, 1], BF16, tag="onescol")
            nc.vector.memset(ones_col[:], 1.0)
            # trigger the exp ACT-table load (~1.3us) during the DMA phase
            # instead of on the first real softmax exp
            warm = small_pool.tile([1, 1], F32, tag="actwarm")
            nc.scalar.activation(warm, ones_col[:1, :1], EXP)
            nc.vector.tensor_copy(
                v_s[:, :, :, DH],
                ones_col[:, :, None].to_broadcast([P, NJ, HPC]),
            )
            ones_f32 = small_pool.tile([1, DH], F32, tag="ones")
            nc.vector.memset(ones_f32[:], 1.0)

            # wo reuses a w_pool slot (freed once the wq reads of phase 1
            # retire); its DMA overlaps with the attention phase.
            wo_s = w_pool.tile([P, NMC, D], F32R, tag="w")
            for kc in range(NMC):
                nc.sync.dma_start(wo_s[:, kc, :], wo_r[kc])

            def emit_q1(m, i):
                isl = slice(i * IC, (i + 1) * IC)
                pq = psum_pp.tile([P, IC], F32, tag="pp")
                for kc in range(NKC):
                    nc.tensor.matmul(
                        pq,
                        lhsT=wq_s[:, kc, m * P:(m + 1) * P],
                        rhs=xt_s[:, kc, isl],
                        start=(kc == 0),
                        stop=(kc == NKC - 1),
                    )
                nc.vector.tensor_copy(qt_s[:, m, isl], pq)

            def emit_k1(m, i):
                isl = slice(i * IC, (i + 1) * IC)
                pk = psum_pp.tile([P, IC], F32, tag="pp")
                for kc in range(NKC):
                    nc.tensor.matmul(
                        pk,
                        lhsT=wk_s[:, kc, m * P:(m + 1) * P],
                        rhs=xt_s[:, kc, isl],
                        start=(kc == 0),
                        stop=(kc == NKC - 1),
                    )
                nc.vector.tensor_copy(kt_s[:, m, isl], pk)

            def emit_qk(m, only_i=None):
                for i in range(NI):
                    if only_i is not None and i != only_i:
                        continue
                    emit_q1(m, i)
                    emit_k1(m, i)

            ots = []
            for i in range(NI):
                ots.append(
                    ot_pool.tile([P, NMC, IC], F32R, tag="ot", name=f"ot_{i}")
                )

            class Block:
                """One (i, pair) attention block in the flat sweep pipeline."""

                def __init__(self, i, pair, hooks):
                    self.i = i
                    self.pair = pair
                    self.hooks = hooks or {}
                    self.isl = slice(i * IC, (i + 1) * IC)
                    self.us = None
                    self.p_ts = [None] * NJ

                def emit_st(self, jc):
                    jsl = slice(jc * P, (jc + 1) * P)
                    p_t = p_pool.tile(
                        [P, 2, IC], BF16, tag="p", name=f"p_{self.i}_{self.pair}_{jc}"
                    )
                    self.p_ts[jc] = p_t
                    st = psum_st.tile([P, 2, IC], F32, tag="st")
                    for half in range(2):
                        hsl = slice(half * DH, (half + 1) * DH)
                        nc.tensor.matmul(
                            st[:, half],
                            lhsT=kt_s[hsl, self.pair, jsl],
                            rhs=qt_s[hsl, self.pair, self.isl],
                            start=True,
                            stop=True,
                        )
                    nc.scalar.activation(p_t, st, EXP)
                    if os.environ.get("ATTN_FUSEMASK", "1") == "1":
                        # one DVE op for both halves; the mask broadcasts
                        # across the half dim with a stride-0 AP
                        nc.vector.tensor_mul(
                            out=p_t, in0=p_t,
                            in1=binm_s[:, jc, None, self.isl].to_broadcast(
                                [P, 2, IC]
                            ),
                        )
                    else:
                        for half in range(2):
                            nc.vector.tensor_mul(
                                out=p_t[:, half], in0=p_t[:, half],
                                in1=binm_s[:, jc, self.isl],
                            )

                def emit_pv(self, jc):
                    if self.us is None:
                        self.us = [
                            psum_u.tile([P, IC], F32, tag="pp", name=f"u_{self.i}_{self.pair}_{h}")
                            for h in range(2)
                        ]
                    for half in range(2):
                        h = 2 * self.pair + half
                        nc.tensor.matmul(
                            self.us[half][: DH + 1],
                            lhsT=v_s[:, jc, h, :],
                            rhs=self.p_ts[jc][:, half],
                            start=(jc == 0),
                            stop=(jc == NJ - 1),
                        )
                    self.p_ts[jc] = None

                def emit_evac(self):
                    mode = os.environ.get("ATTN_EVAC", "scalar")
                    if mode == "0":
                        self.u_sbs = None
                        return
                    # evacuate U' to SBUF: value rows on ScalarE, denominator
                    # row via DVE to a partition-0 tile (reciprocal_approx
                    # misreads nonzero partition offsets on hardware). Frees
                    # the PSUM buffer after these two hops; the normalize
                    # chain then runs from SBUF off the engines that gate the
                    # attention inner loop.
                    self.u_sbs = []
                    self.dens = []
                    for half in range(2):
                        u_sb = us_pool.tile([DH, IC], F32, tag="usb")
                        nc.scalar.copy(u_sb, self.us[half][:DH, :])
                        den = small_pool.tile([1, IC], F32, tag="den")
                        nc.vector.tensor_copy(den, self.us[half][DH:DH + 1, :])
                        if _dbg and self.i == 0 and self.pair == 0:
                            nc.sync.dma_start(dbg_u[half, :DH, :], u_sb)
                            nc.sync.dma_start(dbg_u[half, DH:DH + 1, :], den)
                        self.u_sbs.append(u_sb)
                        self.dens.append(den)

                def emit_chain(self, pe_bcast=False):
                    ot = ots[self.i]
                    for half in range(2):
                        if self.u_sbs is None:
                            # baseline path: normalize straight from PSUM
                            u = self.us[half]
                            rsum = small_pool.tile([1, IC], F32, tag="rsum")
                            nc.vector.tensor_copy(rsum, u[DH:DH + 1, :])
                            rr = small_pool.tile([1, IC], F32, tag="rr")
                            nc.vector.reciprocal_approx_fast(out=rr, in_=rsum)
                            bcs = bc_pool.tile([DH, IC], F32, tag="bcs")
                            nc.gpsimd.partition_broadcast(bcs, rr)
                            nc.vector.tensor_mul(
                                out=ot[half * DH:(half + 1) * DH, self.pair, :],
                                in0=u[:DH, :],
                                in1=bcs,
                            )
                            continue
                        u_sb = self.u_sbs[half]
                        rr = small_pool.tile([1, IC], F32, tag="rr")
                        nc.vector.reciprocal_approx_fast(
                            out=rr, in_=self.dens[half]
                        )
                        if _dbg and self.i == 0 and self.pair == 0:
                            nc.sync.dma_start(dbg_rr[half], rr)
                        bcs = bc_pool.tile([DH, IC], F32, tag="bcs")
                        if pe_bcast:
                            # tail blocks: Pool's serial chain would gate the
                            # final projection; broadcast via a K=1 matmul
                            # and normalize on DVE instead.
                            # plain fp32 matmul: slower per row but runs on
                            # the otherwise-idle tail PE, and avoids the
                            # fp32r rounded-producer requirement
                            bcp = psum_pp.tile([DH, IC], F32, tag="pp")
                            nc.tensor.matmul(
                                bcp, lhsT=ones_f32, rhs=rr,
                                start=True, stop=True,
                            )
                            nc.scalar.copy(bcs, bcp)
                            nc.vector.tensor_mul(
                                out=ot[half * DH:(half + 1) * DH, self.pair, :],
                                in0=u_sb[:DH, :],
                                in1=bcs,
                            )
                        else:
                            nc.gpsimd.partition_broadcast(bcs, rr)
                            if _dbg and self.i == 0 and self.pair == 0:
                                nc.sync.dma_start(dbg_bc[half], bcs)
                            norm_eng = (
                                nc.gpsimd
                                if os.environ.get("ATTN_NORM_ENG", "dve") == "pool"
                                else nc.vector
                            )
                            norm_eng.tensor_mul(
                                out=ot[half * DH:(half + 1) * DH, self.pair, :],
                                in0=u_sb[:DH, :],
                                in1=bcs,
                            )

            def sweep(blocks):
                """Flat software pipeline over all (block, jc) steps: ST runs
                one step ahead of PV so exp+mask latency is always covered,
                including across block boundaries. The normalize chain of a
                finished block is emitted two steps later so its DVE/Pool ops
                queue behind the next block's first masks rather than
                stalling them."""
                delay = int(os.environ.get("ATTN_CHAINDELAY", "2"))
                pe_mode = os.environ.get("ATTN_PEBC", "1")
                pe_bc = pe_mode in ("1", "all")
                pe_all = pe_mode == "all"
                steps = [(b, jc) for b in blocks for jc in range(NJ)]
                pending = []
                for k, (b, jc) in enumerate(steps):
                    if jc in b.hooks:
                        b.hooks[jc]()
                    b.emit_st(jc)
                    if k > 0:
                        pb, pjc = steps[k - 1]
                        pb.emit_pv(pjc)
                        if pjc == NJ - 1:
                            pb.emit_evac()
                            pending.append((k + delay, pb))
                    if pending and pending[0][0] <= k:
                        pending.pop(0)[1].emit_chain(pe_bcast=pe_all)
                b, jc = steps[-1]
                b.emit_pv(jc)
                b.emit_evac()
                for _, pb in pending:
                    pb.emit_chain(pe_bcast=pe_all)
                b.emit_chain(pe_bcast=pe_bc)

            def emit_v(jc):
                pv = psum_pp.tile([P, IPC], F32, tag="pp")
                for kc in range(NKC):
                    nc.tensor.matmul(
                        pv,
                        lhsT=xt_s[:, kc, jc * P:(jc + 1) * P],
                        rhs=wv_s[:, kc, :],
                        start=(kc == 0),
                        stop=(kc == NKC - 1),
                    )
                nc.vector.tensor_copy(
                    v_s[:, jc, :, :DH], pv.rearrange("p (h d) -> p h d", h=HPC)
                )

            def emit_proj_chunk(i, do, stg_eng):
                isl = slice(i * IC, (i + 1) * IC)
                pr = psum_pp.tile([P, IC], F32, tag="pp")
                for kc in range(NMC):
                    nc.tensor.matmul(
                        pr,
                        lhsT=_r(wo_s[:, kc, do * P:(do + 1) * P]),
                        rhs=_r(ots[i][:, kc, :]),
                        start=(kc == 0),
                        stop=(kc == NMC - 1),
                    )
                stg = stage_pool.tile([P, IC], F32, tag="stg")
                if stg_eng == "dve":
                    nc.vector.tensor_copy(stg, pr)
                else:
                    nc.scalar.copy(stg, pr)
                nc.sync.dma_start(outT_r[do][:, isl], stg)

            # ---------------- schedule ----------------
            # V projection first (xt+wv DMAs land first), then QK for the
            # first block, then i-major attention sweeps. NOTE: kt's "i"
            # slices are KEY POSITIONS — every attention block reads the full
            # kt row of its pair — so K must be complete per pair before the
            # pair's first block; only Q is split by query chunk.
            for jc in range(NJ):
                emit_v(jc)
            emit_q1(0, 0)
            emit_k1(0, 0)
            emit_k1(0, 1)

            # i0 sweep: prefetch Q(pair+1, i0), full K(pair+1) and Q(pair, i1)
            # spread across four hook points per block.
            # i1 sweep: interleave the i0 output projection (ots[0] is only
            # complete after the i0-p3 normalize chain, so start at pair 1).
            blocks = []
            for pair in range(NMC):
                if pair + 1 < NMC:
                    hooks = {
                        1: lambda m=pair + 1: emit_q1(m, 0),
                        3: lambda m=pair + 1: emit_k1(m, 0),
                        5: lambda m=pair + 1: emit_k1(m, 1),
                        7: lambda m=pair: emit_q1(m, 1),
                    }
                else:
                    hooks = {
                        1: lambda m=pair: emit_q1(m, 1),
                    }
                blocks.append(Block(0, pair, hooks))
            # proj(0) chunks spread over all four i1 blocks (2 per block,
            # one extra on p1/p2); a chunk's pair-0..2 matmuls can run before
            # the i0-p3 normalize chain lands, so even i1-p0 gets PE work
            proj_sched = {0: [], 1: [(1, 0), (3, 1), (5, 2)],
                          2: [(1, 3), (3, 4), (5, 5)], 3: [(1, 6), (3, 7)]}
            for pair in range(NMC):
                hooks = {}
                for jc_h, d in proj_sched[pair]:
                    hooks[jc_h] = lambda d=d: emit_proj_chunk(0, d, "dve")
                blocks.append(Block(1, pair, hooks))
            sweep(blocks)

            if _dbg:
                for i in range(NI):
                    nc.sync.dma_start(dbg_ot[i], ots[i].bitcast(F32))

            # tail: i1 projection, staging split across ACT and DVE
            for do in range(NDO):
                emit_proj_chunk(1, do, "dve" if do % 2 else "act")

    nc.compile()
    return nc


_nc_cache = {}


def _get_nc(loop=1):
    if loop not in _nc_cache:
        _nc_cache[loop] = _build(loop)
    return _nc_cache[loop]


_last_results = [None]
_last_in_maps = [None]


def kernel(x, mask, Wq, Wk, Wv, Wo, bo):
    x = np.asarray(x, dtype=np.float32)
    mask = np.asarray(mask)
    Wq = np.asarray(Wq, dtype=np.float32)
    Wk = np.asarray(Wk, dtype=np.float32)
    Wv = np.asarray(Wv, dtype=np.float32)
    Wo = np.asarray(Wo, dtype=np.float32)
    bo = np.asarray(bo, dtype=np.float32)

    nc = _get_nc()
    in_maps = []
    for c in range(NCORES):
        b, g = divmod(c, 2)
        gsl = slice(g * IPC, (g + 1) * IPC)
        keep = (mask[b, 0] == 0).T
        in_maps.append(
            {
                "xT": np.ascontiguousarray(x[b].T.astype(ml_dtypes.bfloat16)),
                "wq": np.ascontiguousarray((Wq[:, gsl] * np.float32(SCALE)).astype(ml_dtypes.bfloat16)),
                "wk": np.ascontiguousarray(Wk[:, gsl].astype(ml_dtypes.bfloat16)),
                "wv": np.ascontiguousarray(Wv[:, gsl].astype(ml_dtypes.bfloat16)),
                "wo": np.ascontiguousarray(Wo[gsl, :]),
                "binmT": np.ascontiguousarray(keep.astype(ml_dtypes.bfloat16)),
            }
        )
    _last_in_maps[0] = in_maps
    res = run_bass_kernel_spmd(nc, in_maps, core_ids=list(range(NCORES)))
    _last_results[0] = res
    outs = [r["outT"] for r in res.results]
    out = np.empty((B, N, D), np.float32)
    for b in range(B):
        out[b] = (outs[2 * b] + outs[2 * b + 1]).T + bo
    return out

